# revision 110
# baseline (speedup 1.0000x reference)
"""3-layer GCN (PyG GCNConv semantics) on 8 Trainium2 NeuronCores.

Design (v5, 0.898 ms TimelineSim vs 1.62 ms v2):
  - The first 36 blocks' region-1 sweeps are deferred to the end of phase
    A (region-0 partial stashed alone, deferred sweep accumulates into the
    stash): region 1 of the table is first gathered ~33us into the layer,
    absorbing the previous layer's quarter-1 AllGather latency (it drains
    the serial collective device ~12us after that layer ends). Cuts the
    two ~42us layer-boundary stalls for one extra DVE op per deferred
    block. Expand q1 is emitted in-layer at block 93 (q0 at 78): late
    enough that its SP SEQ park is short, early enough to beat
    end-of-layer dispatch (both alternatives measured slower).

From v4 (0.935 ms):
  - Sel one-hot masks built KSEL=32 tiles per DVE instruction: d-major
    interleaved layout sel[p, d*K+j] compared against a 0-stride broadcast
    of the dstloc columns. The broadcast sits on a middle AP dim so the
    2-byte packed DVE fast mode still applies: ~69ns/tile vs 93, and 32x
    fewer DVE instructions (fewer sem hops, less PE Ldweights wait time).
    The per-tile matmul lhsT is a stride-K column view of the group mask
    (verified bit-exact through the device path).

Inherited from v3 (1.21 ms):
  - Table rows quarter-mapped: region r of the gather table = quarter r of
    every core's shard. The inter-layer AllGather is split into 4 per-region
    collectives; each next-layer expand DMA is chained behind BOTH its
    quarter's collective AND a g16 store ~30 block-epilogues later, so it
    reaches the SP sequencer with its wait already satisfied (a parked DMA
    wait head-blocks all later SP DMA dispatches for up to 55us). The
    layer-1 table arrives pre-padded from the host: no startup expands.
  - EGATHER=32: 64B gather descriptors at the 7ns DMA floor (raw
    InstDMAGatherAnt; 256B-multiple restriction is transpose-only;
    verified bit-exact on hardware).
  - Two-phase layers over groups of GBLK=6 dst blocks: phase A sweeps src
    regions {0,1} region-minor, each block's partial accumulating in one
    PSUM bank, then one DVE op stashes dinv*gpA+gown to SBUF (bf16);
    phase B sweeps regions {2,3} the same way and finishes with the
    epilogue. Regions 2,3 of the table are first needed ~50% into the
    layer, giving the previous layer's late collectives half a layer of
    slack; PSUM residency removes all per-region acc copy/add DVE ops.
  - Epilogue: xb = dinv*gpB + stash on DVE, PE transpose, relu(x+b) and
    the g16/gown rescales on the idle Activation engine; g16 lives in a
    persistent buffer that doubles as next layer's gown source.
  - Deep pools (sel=4x32 tiles, msg=10 calls) so the sel stream rides ahead
    of the PE.
  - Remaining bottlenecks: DVE busy ~860us (88%), PE SEQ issue rate
    (~80ns/Ldweights, ~100% held), and 2x ~42us layer-boundary stalls from
    the quarter-0/1 collectives serializing on COLLECTIVE_CORES (55us
    each) past the layer end. Going lower needs half-quarter collectives
    (restructured expand APs) or fewer tiles; int16 gather indices cap
    regions at 32K rows (interp asserts idx >= -1, so no signed trick).
"""

import numpy as np
import ml_dtypes

P = 128
D = 32             # feature width
ELEM = 128         # table row stride: 128 bf16 = 256B
EGATHER = 32       # gathered elems per row (32 -> 64B descriptors @ 7ns floor)
SAFE_BARRIERS = False  # True: barrier-fenced boundaries (debug), no sem pipeline
MAX_CALL = 8192    # max indices per dma_gather call
NCORES = 8
NREG = 4           # src index regions (int16 reach); also collective quarters
GBLK = 6           # dst blocks per PSUM-resident accumulation group
KSEL = 64          # sel masks built per DVE instruction (batched is_equal)

BF16 = ml_dtypes.bfloat16


# ----------------------------------------------------------------- host side

def _preprocess(N, edge_index):
    """Edge structure only (no x-dependent data): cacheable."""
    assert N % NCORES == 0
    NSH = N // NCORES                       # dst nodes per core
    NLOC = ((NSH + P - 1) // P) * P         # padded to blocks of 128
    NB = NLOC // P
    assert NLOC % NREG == 0
    QS = NLOC // NREG                       # quarter size (shard rows)
    TBL = NLOC * NCORES                     # total table rows
    RSPAN = TBL // NREG                     # table rows per region
    assert RSPAN <= 32767

    src = np.asarray(edge_index[0], dtype=np.int64)
    dst = np.asarray(edge_index[1], dtype=np.int64)
    deg = np.bincount(dst, minlength=N).astype(np.float64) + 1.0
    dinv = (1.0 / np.sqrt(deg)).astype(np.float32)

    core_of = dst // NSH

    # per-core permutation (in-degree desc within shard); table row of node
    # (core c, slot s): quarter q = s // QS -> q*RSPAN + c*QS + s%QS
    perms, invperms = [], []
    g2t = np.empty(N, np.int64)
    dcnt_all = np.bincount(dst, minlength=N) + 1   # incl self-loop
    for c in range(NCORES):
        cnt = dcnt_all[c * NSH:(c + 1) * NSH]
        perm = np.argsort(-cnt, kind="stable")     # slot -> local node
        inv = np.empty(NSH, np.int64)
        inv[perm] = np.arange(NSH)
        perms.append(perm)
        invperms.append(inv)
        q = inv // QS
        g2t[c * NSH:(c + 1) * NSH] = q * RSPAN + c * QS + inv % QS

    # per-core edge lists (NO self-loops); keyed by (region, dst-block)
    per_core = []
    for c in range(NCORES):
        m = core_of == c
        s_c = src[m]
        dslot = invperms[c][dst[m] - c * NSH]
        stid = g2t[s_c]
        reg = stid // RSPAN
        blk = dslot // P
        per_core.append((stid, dslot, reg, blk))

    # common tile structure: tiles_rb[r, b] = max over cores
    counts = np.zeros((NCORES, NREG, NB), np.int64)
    for c in range(NCORES):
        _, _, reg, blk = per_core[c]
        np.add.at(counts[c], (reg, blk), 1)
    tiles_rb = np.maximum((counts.max(axis=0) + P - 1) // P, 1)  # [NREG, NB]

    # Two-phase, group-of-G ordering. Phase A sweeps regions {0,1} over
    # groups of G blocks (region-minor); each block's partial lives in one
    # PSUM bank for its group sweep, then one DVE op stashes
    # dinv*partial+gown to SBUF. Phase B sweeps regions {2,3} the same way
    # and finishes with the epilogue.
    #
    # The first DEFG blocks' region-1 sweeps are DEFERRED to the end of
    # phase A (their region-0 partial is stashed on its own, and the
    # deferred region-1 sweep accumulates into the stash with one extra
    # DVE op per block). Region 1 of the table is then first gathered
    # ~DEFG*region-tiles into the layer instead of ~one group in - enough
    # slack for the previous layer's quarter-1 AllGather (which drains the
    # serial collective device only ~20us after that layer ends) to land
    # without stalling the pipe. Regions 2,3 are first needed at ~50%.
    #
    # sess[(r, b)] = (session_start, session_stop, stop_kind) where
    # stop_kind 0 = stash-write, 1 = stash-accumulate, 2 = epilogue.
    DEFG = 7 * GBLK
    tile_meta = []        # (region, block, bucket_first, bucket_last)
    sess = {}

    def emit_bucket(r, b):
        nt = int(tiles_rb[r, b])
        for i in range(nt):
            tile_meta.append((r, b, i == 0, i == nt - 1))

    for g0 in range(0, DEFG, GBLK):          # part 1: r0 only, stash-write
        for b in range(g0, g0 + GBLK):
            emit_bucket(0, b)
            sess[(0, b)] = (True, True, 0)
    for g0 in range(DEFG, NB, GBLK):         # part 2: r0+r1, stash-write
        for r in (0, 1):
            for b in range(g0, min(g0 + GBLK, NB)):
                emit_bucket(r, b)
                sess[(r, b)] = (r == 0, r == 1, 0)
    for g0 in range(0, DEFG, GBLK):          # part 3: deferred r1, stash-acc
        for b in range(g0, g0 + GBLK):
            emit_bucket(1, b)
            sess[(1, b)] = (True, True, 1)
    for g0 in range(0, NB, GBLK):            # phase B: r2+r3, epilogue
        for r in (2, 3):
            for b in range(g0, min(g0 + GBLK, NB)):
                emit_bucket(r, b)
                sess[(r, b)] = (r == 2, r == 3, 2)
    T = len(tile_meta)

    # gather calls: chunk tile stream, never crossing region boundaries
    calls = []            # (region, tile_start, ntiles)
    t0 = 0
    while t0 < T:
        r = tile_meta[t0][0]
        nt = 1
        while (t0 + nt < T and tile_meta[t0 + nt][0] == r
               and nt < MAX_CALL // P):
            nt += 1
        calls.append((r, t0, nt))
        t0 += nt

    idx_cols = sum(cl[2] * P // 16 for cl in calls)
    # per-call idx-buffer assignment: the idx table is split into NREG
    # SBUF tiles by stream position (preload pipelining); assignment is
    # monotone and calls never straddle a split.
    call_sb = []
    for (r, t0c, nt) in calls:
        sbi = min(NREG - 1, NREG * t0c // T)
        if call_sb and sbi < call_sb[-1]:
            sbi = call_sb[-1]
        call_sb.append(sbi)
    sb_col_base = [0] * (NREG + 1)
    col = 0
    for ci, (r, t0c, nt) in enumerate(calls):
        col += nt * P // 16
        sb_col_base[call_sb[ci] + 1] = col
    idx_all = np.zeros((NCORES, 16, idx_cols), np.int16)
    dl_all = np.full((NCORES, P, T), 255.0, np.float32)   # 255 = dead slot

    pos = {}
    for ti, (r, b, gf, _gl) in enumerate(tile_meta):
        if gf:
            pos[(r, b)] = ti

    for c in range(NCORES):
        stid, dsl, reg, blk = per_core[c]
        eidx = np.zeros((T, P), np.int64)
        dloc = np.full((T, P), 255, np.int64)
        for ti, (r, _b, _f, _l) in enumerate(tile_meta):
            eidx[ti, :] = r * RSPAN                # any finite row in region
        key = reg * NB + blk
        order = np.argsort(key, kind="stable")
        ks = key[order]
        st_ids = stid[order]
        dls = dsl[order]
        uq, starts = np.unique(ks, return_index=True)
        starts = list(starts) + [len(ks)]
        for u_i, k in enumerate(uq):
            r, b = int(k) // NB, int(k) % NB
            lo, hi = starts[u_i], starts[u_i + 1]
            n = hi - lo
            ti = pos[(r, b)] + np.arange(n) // P
            lane = np.arange(n) % P
            eidx[ti, lane] = st_ids[lo:hi]
            dloc[ti, lane] = dls[lo:hi] % P
        dl_all[c] = dloc.T.astype(np.float32)
        col0 = 0
        for (r, t0c, nt) in calls:
            flat = (eidx[t0c:t0c + nt].reshape(-1) - r * RSPAN).astype(np.int16)
            ncol = nt * P // 16
            idx_all[c, :, col0:col0 + ncol] = flat.reshape(ncol, 16).T
            col0 += ncol

    struct = {
        "N": N, "NSH": NSH, "NLOC": NLOC, "NB": NB, "QS": QS, "TBL": TBL,
        "RSPAN": RSPAN, "tile_meta": tile_meta, "calls": calls, "T": T,
        "idx_cols": idx_cols,
        "call_sb": call_sb, "sb_col_base": sb_col_base, "sess": sess,
    }
    per_core_data = {"idx": idx_all, "dstloc": dl_all}
    return struct, per_core_data, dinv, perms


def _host_tables(x1, W11, dinv, perms, struct):
    """layer-1 compact table t1c = (dinv*x1) @ W11, quarter-mapped, bf16."""
    NSH, NLOC, QS, RSPAN = (struct["NSH"], struct["NLOC"], struct["QS"],
                            struct["RSPAN"])
    g1 = (dinv[:, None] * np.asarray(x1, np.float32)) @ np.asarray(W11, np.float32)
    t1 = np.zeros((struct["TBL"], D), np.float32)
    for c in range(NCORES):
        gperm = np.zeros((NLOC, D), np.float32)
        gperm[:NSH] = g1[c * NSH + perms[c]]
        # shard slot s -> row (s//QS)*RSPAN + c*QS + s%QS
        t1.reshape(NREG, NCORES, QS, D)[:, c] = gperm.reshape(NREG, QS, D)
    return t1.astype(BF16)


# --------------------------------------------------------------- device side

def _raw_dma_gather(g, out_ap, in_ap, idxs_ap, num_idxs, elem_size, elem_step,
                    queue_num=0):
    """BassGpSimd.dma_gather minus the %256 elem_size restriction."""
    import concourse.bass as bass
    import concourse.mybir as mybir
    from concourse import ap_utils
    assert idxs_ap.dtype == mybir.dt.int16
    assert in_ap.dtype == out_ap.dtype
    assert ap_utils.ap_is_contiguous(in_ap.ap[1:])
    assert ap_utils.ap_is_contiguous(out_ap.ap[1:])
    assert ap_utils.ap_is_contiguous(idxs_ap.ap[1:])
    assert in_ap.ap[-1][1] == out_ap.ap[-1][1] == elem_size
    assert in_ap.ap[0][0] == elem_step
    stride_bytes = elem_step * mybir.dt.size(in_ap.dtype)
    stride_bytes_256 = stride_bytes // 256
    assert stride_bytes % 256 == 0 and stride_bytes_256 < 256
    _in_ap = g.lower_ap_dma(in_ap, for_custom_bir_dma=True)
    _idxs_ap = g.lower_ap(idxs_ap)
    _out_ap = g.lower_ap(out_ap)
    return g.add_instruction(
        mybir.InstDMAGatherAnt(
            name=g.bass.get_next_instruction_name(),
            ins=[*_in_ap, _idxs_ap, g.lower_val_access(g.to_reg(num_idxs))],
            outs=[_out_ap],
            transpose=False,
            num_idxs=num_idxs,
            elem_size=elem_size,
            stride_bytes_256=stride_bytes_256,
            gen_mode=0,
            single_packet=False,
            queue_num=queue_num,
        )
    )


def _build_program(struct, fc_b_val):
    import concourse.bacc as bacc
    import concourse.mybir as mybir
    import concourse.tile as tile
    from concourse.library_config import mlp
    from concourse.masks import make_identity

    NB, TBL, RSPAN, QS = (struct["NB"], struct["TBL"], struct["RSPAN"],
                          struct["QS"])
    NLOC = struct["NLOC"]
    T = struct["T"]
    tile_meta = struct["tile_meta"]
    calls = struct["calls"]
    idx_cols = struct["idx_cols"]
    sess = struct["sess"]

    nc = bacc.Bacc(None, target_bir_lowering=False, num_swdge_queues=4)
    dt = mybir.dt

    # layer-1 table arrives pre-padded to the 256B-row gather layout (host
    # pads for free): no startup expand DMAs at all.
    tpad = nc.declare_dram_parameter("t1pad", [TBL, ELEM], dt.bfloat16,
                                     isOutput=False)
    idx = nc.declare_dram_parameter("idx", [P, idx_cols], dt.int16, isOutput=False)
    dstloc = nc.declare_dram_parameter("dstloc", [P, T], dt.bfloat16, isOutput=False)
    dinvb = nc.declare_dram_parameter("dinvb", [P, NB], dt.float32, isOutput=False)
    g1own = nc.declare_dram_parameter("g1own", [P, NB * D], dt.bfloat16, isOutput=False)
    bcol = nc.declare_dram_parameter("bcol", [D, 3], dt.float32, isOutput=False)
    w2 = nc.declare_dram_parameter("w2", [D, D], dt.bfloat16, isOutput=False)
    w3 = nc.declare_dram_parameter("w3", [D, D], dt.bfloat16, isOutput=False)
    fcw = nc.declare_dram_parameter("fcw", [D, 1], dt.bfloat16, isOutput=False)
    iota = nc.declare_dram_parameter("iota", [P, P * KSEL], dt.bfloat16,
                                     isOutput=False)
    y = nc.declare_dram_parameter("y", [NLOC, 1], dt.float32, isOutput=True)

    g2c = nc.dram_tensor("g2c", [NLOC, D], dt.bfloat16)
    g3c = nc.dram_tensor("g3c", [NLOC, D], dt.bfloat16)
    t2c = nc.dram_tensor("t2c", [TBL, D], dt.bfloat16, addr_space="Shared")
    t3c = nc.dram_tensor("t3c", [TBL, D], dt.bfloat16, addr_space="Shared")

    rg = [list(range(NCORES))]
    CHUNKS = MAX_CALL // P

    # blocks that must be written before quarter q's collective fires
    nb_q = [-(-((q + 1) * QS) // P) for q in range(NREG)]     # cumulative

    from concourse.bass import _add_dep_helper

    with tile.TileContext(nc) as tc:
        with (
            tc.tile_pool(name="const", bufs=1) as cpool,
            tc.tile_pool(name="msg", bufs=(10 if EGATHER <= 32 else 5)) as mpool,
            tc.tile_pool(name="sel", bufs=2) as spool,
            tc.tile_pool(name="ep", bufs=16) as epool,
            tc.tile_pool(name="stash", bufs=2) as stashpool,
            tc.tile_pool(name="gsb", bufs=2) as gsbpool,
            tc.tile_pool(name="gp", bufs=GBLK, space="PSUM") as gpool,
            tc.tile_pool(name="eppsum", bufs=1, space="PSUM") as eppool,
        ):
            nc.gpsimd.load_library(mlp)
            call_sb = struct["call_sb"]
            sb_col_base = struct["sb_col_base"]
            idx_sbs = [cpool.tile([P, sb_col_base[i + 1] - sb_col_base[i]],
                                  dt.int16, name=f"idxsb{i}")
                       for i in range(NREG)]
            dl_sb = cpool.tile([P, T], dt.bfloat16)
            dinv_sb = cpool.tile([P, NB], dt.float32)
            g1own_sb = cpool.tile([P, NB * D], dt.bfloat16)
            bcol_sb = cpool.tile([D, 3], dt.float32)
            w2_sb = cpool.tile([D, D], dt.bfloat16)
            w3_sb = cpool.tile([D, D], dt.bfloat16)
            fcw_sb = cpool.tile([D, 1], dt.bfloat16)
            iota_sb = cpool.tile([P, P * KSEL], dt.bfloat16)
            ident = cpool.tile([P, P], dt.bfloat16)

            # startup order: what the first gathers need comes first. The
            # first gather call and sel batches only touch the leading
            # idx/dstloc columns, so those land as small chunks before the
            # bulk loads (slice-level deps let consumers start early).
            nc.sync.dma_start(out=iota_sb[:], in_=iota[:])
            SP0 = min(512, sb_col_base[1])
            nc.sync.dma_start(out=idx_sbs[0][:, :SP0], in_=idx[:, :SP0])
            nc.sync.dma_start(out=dl_sb[:, :256], in_=dstloc[:, :256])
            nc.sync.dma_start(out=idx_sbs[0][:, SP0:],
                              in_=idx[:, SP0:sb_col_base[1]])
            nc.sync.dma_start(out=dl_sb[:, 256:], in_=dstloc[:, 256:])
            make_identity(nc, ident[:])
            # layer-1 table is pre-padded on host: no startup expands
            expands = [None] * (3 * NREG)
            idx_loads = [None] * NREG
            for i in range(1, NREG):
                idx_loads[i] = nc.sync.dma_start(
                    out=idx_sbs[i][:],
                    in_=idx[:, sb_col_base[i]:sb_col_base[i + 1]])
            nc.sync.dma_start(out=dinv_sb[:], in_=dinvb[:])
            nc.sync.dma_start(out=g1own_sb[:], in_=g1own[:])
            nc.sync.dma_start(out=bcol_sb[:], in_=bcol[:])
            nc.sync.dma_start(out=w2_sb[:], in_=w2[:])
            nc.sync.dma_start(out=w3_sb[:], in_=w3[:])
            nc.sync.dma_start(out=fcw_sb[:], in_=fcw[:])
            if SAFE_BARRIERS:
                tc.strict_bb_all_engine_barrier()

            tabs = [None, t2c, t3c]
            gouts = [g2c, g3c, None]
            wnext = [w2_sb, w3_sb, None]

            def emit_epilogue(L, b, gp, gcur, stash):
                # conv_out = dinv*gpB + (dinv*gpA + gown) [phase-A stash]
                xb = epool.tile([P, D], dt.bfloat16, name=f"x{L}_{b}", tag="xb")
                nc.vector.scalar_tensor_tensor(
                    out=xb[:], in0=gp[:],
                    scalar=dinv_sb[:, b:b + 1],
                    in1=stash[:, b * D:(b + 1) * D],
                    op0=mybir.AluOpType.mult, op1=mybir.AluOpType.add)
                xT = eppool.tile([D, P], dt.bfloat16, name=f"xT{L}_{b}", tag="xT")
                nc.tensor.transpose(out=xT[:], in_=xb[:], identity=ident[:])
                xT_sb = epool.tile([D, P], dt.bfloat16,
                                   name=f"xTs{L}_{b}", tag="xTs")
                nc.scalar.activation(
                    out=xT_sb[:], in_=xT[:],
                    func=mybir.ActivationFunctionType.Relu,
                    bias=bcol_sb[:, L:L + 1], scale=1.0)
                if L < 2:
                    h = eppool.tile([P, D], dt.float32,
                                    name=f"h{L}_{b}", tag="h")
                    nc.tensor.matmul(out=h[:], lhsT=xT_sb[:],
                                     rhs=wnext[L][:], start=True, stop=True)
                    # g16 = dinv*h: collective input. The next layer's
                    # self-loop term gown = dinv*g16 goes to the persistent
                    # gown buffer via one more cheap Act rescale.
                    nc.scalar.activation(
                        out=gcur[:, b * D:(b + 1) * D], in_=h[:],
                        func=mybir.ActivationFunctionType.Copy,
                        scale=dinv_sb[:, b:b + 1])
                    gdma = nc.sync.dma_start(
                        out=gouts[L][b * P:(b + 1) * P, :],
                        in_=gcur[:, b * D:(b + 1) * D])
                    g16_dmas[b] = gdma
                    nc.scalar.activation(
                        out=gownbuf[:, b * D:(b + 1) * D],
                        in_=gcur[:, b * D:(b + 1) * D],
                        func=mybir.ActivationFunctionType.Copy,
                        scale=dinv_sb[:, b:b + 1])
                else:
                    yp = eppool.tile([P, 1], dt.float32,
                                     name=f"yp{b}", tag="h")
                    nc.tensor.matmul(out=yp[:], lhsT=xT_sb[:],
                                     rhs=fcw_sb[:], start=True, stop=True)
                    y_sb = epool.tile([P, 1], dt.float32,
                                      name=f"ys{b}", tag="g")
                    nc.vector.tensor_scalar(
                        out=y_sb[:], in0=yp[:],
                        scalar1=float(fc_b_val), scalar2=None,
                        op0=mybir.AluOpType.add)
                    nc.sync.dma_start(out=y[b * P:(b + 1) * P, :], in_=y_sb[:])

            gownbuf = gsbpool.tile([P, NB * D], dt.bfloat16,
                                   name="gownbuf", tag="gown")
            for L in range(3):
                next_q = 0            # next quarter collective to emit
                ccs = [None] * NREG
                g16_dmas = [None] * NB
                gp_of = {}            # block -> live PSUM accumulator tile
                gown = g1own_sb if L == 0 else gownbuf
                stash = stashpool.tile([P, NB * D], dt.bfloat16,
                                       name=f"stash{L}", tag="stash")
                gcur = (gsbpool.tile([P, NB * D], dt.bfloat16,
                                     name=f"gcur{L}", tag="gsb")
                        if L < 2 else None)
                icol = 0
                for ci, (r, t0c, nt) in enumerate(calls):
                    dep_exp = None if SAFE_BARRIERS else expands[NREG * L + r]
                    nidx = nt * P
                    ncol = nidx // 16
                    sbi = call_sb[ci]
                    lcol = icol - sb_col_base[sbi]
                    msg = mpool.tile([P, CHUNKS * EGATHER], dt.bfloat16,
                                     name=f"msg{L}_{ci}", tag="msg")
                    if EGATHER == ELEM:
                        gi = nc.gpsimd.dma_gather(
                            msg[:, : nt * EGATHER].rearrange(
                                "p (c e) -> p c e", e=EGATHER),
                            tpad[r * RSPAN:(r + 1) * RSPAN, :],
                            idx_sbs[sbi][:, lcol:lcol + ncol],
                            nidx, nidx, EGATHER,
                            single_packet=False, queue_num=ci % 4)
                    else:
                        gi = _raw_dma_gather(
                            nc.gpsimd,
                            msg[:, : nt * EGATHER].rearrange(
                                "p (c e) -> p c e", e=EGATHER),
                            tpad[r * RSPAN:(r + 1) * RSPAN, 0:EGATHER],
                            idx_sbs[sbi][:, lcol:lcol + ncol],
                            nidx, EGATHER, ELEM, queue_num=ci % 4)
                    if dep_exp is not None:
                        _add_dep_helper(gi.ins, dep_exp.ins, sync=True,
                                        reason="gather after region expand")
                    if L == 0 and ci in (2, 4, 6):
                        # startup: later idx loads are not needed for a
                        # while; chain them behind early gathers so they
                        # don't hog the DMA engines before the pipeline is
                        # primed.
                        _add_dep_helper(idx_loads[ci // 2].ins, gi.ins,
                                        sync=True, reason="delay idx load")
                    icol += ncol
                    msg3 = msg[:].rearrange("p (c e) -> p c e", e=EGATHER)
                    for c in range(nt):
                        t_glob = t0c + c
                        _r, b, gfst, glst = tile_meta[t_glob]
                        # sel masks built KSEL tiles per DVE instruction in
                        # d-major interleave: sel_g[p, d*kb + j] is mask
                        # element (edge p, dst d) of tile t0+j. The 0-stride
                        # broadcast of dl sits on a middle AP dim, keeping
                        # the 2-byte packed fast mode: ~74ns/tile vs 93,
                        # and 8x fewer DVE instructions.
                        if t_glob % KSEL == 0:
                            kb = min(KSEL, T - t_glob)
                            sel_g = spool.tile([P, P * kb], dt.bfloat16,
                                               name=f"sel{L}_{t_glob}",
                                               tag="sel")
                            i3 = iota_sb[:].rearrange(
                                "p (d k) -> p d k", k=KSEL)[:, :, :kb]
                            d3 = dl_sb[:, t_glob:t_glob + kb].rearrange(
                                "p (o k) -> p o k", o=1).broadcast_to(
                                (P, P, kb))
                            nc.vector.tensor_tensor(
                                out=sel_g[:].rearrange(
                                    "p (d k) -> p d k", k=kb),
                                in0=i3, in1=d3,
                                op=mybir.AluOpType.is_equal)
                            sel_view = sel_g[:].rearrange(
                                "p (d k) -> p d k", k=kb)
                        sstart, sstop, skind = sess[(_r, b)]
                        if sstart and gfst:
                            gp_of[b] = gpool.tile([P, D], dt.float32,
                                                  name=f"gp{L}_{t_glob}",
                                                  tag="gp")
                        gp = gp_of[b]
                        nc.tensor.matmul(
                            out=gp[:],
                            lhsT=sel_view[:, :, t_glob % KSEL],
                            rhs=msg3[:, c, 0:D],
                            start=bool(sstart and gfst),
                            stop=bool(sstop and glst),
                            skip_group_check=True)
                        if glst and sstop:
                            del gp_of[b]
                            if skind == 0:
                                # stash dinv*gp + gown to SBUF
                                nc.vector.scalar_tensor_tensor(
                                    out=stash[:, b * D:(b + 1) * D], in0=gp[:],
                                    scalar=dinv_sb[:, b:b + 1],
                                    in1=gown[:, b * D:(b + 1) * D],
                                    op0=mybir.AluOpType.mult,
                                    op1=mybir.AluOpType.add)
                            elif skind == 1:
                                # deferred region-1 partial: accumulate
                                # into the existing stash
                                nc.vector.scalar_tensor_tensor(
                                    out=stash[:, b * D:(b + 1) * D], in0=gp[:],
                                    scalar=dinv_sb[:, b:b + 1],
                                    in1=stash[:, b * D:(b + 1) * D],
                                    op0=mybir.AluOpType.mult,
                                    op1=mybir.AluOpType.add)
                            else:
                                # block b fully aggregated: epilogue inline so
                                # quarter collectives fire during phase B
                                emit_epilogue(L, b, gp, gcur, stash)
                                if (L < 2 and next_q < NREG
                                        and b + 1 == nb_q[next_q]
                                        and not SAFE_BARRIERS):
                                    cc = nc.gpsimd.collective_compute(
                                        "AllGather", mybir.AluOpType.bypass,
                                        replica_groups=rg,
                                        ins=[gouts[L][next_q * QS:
                                                      (next_q + 1) * QS, :]],
                                        outs=[tabs[L + 1][next_q * RSPAN:
                                                          (next_q + 1) * RSPAN, :]])
                                    lo = 0 if next_q == 0 else nb_q[next_q - 1]
                                    for bb in range(lo, nb_q[next_q]):
                                        _add_dep_helper(
                                            cc.ins, g16_dmas[bb].ins, sync=True,
                                            reason="collective after quarter g16")
                                    ccs[next_q] = cc
                                    next_q += 1
                                # quarter 0 of the next layer's table is
                                # needed right at the next layer's start:
                                # emit its expand in-layer at ~80% of the
                                # block stream (cc0 has just completed),
                                # paced by this block's g16 so the SP SEQ
                                # never parks on it. Quarter 1's collective
                                # only drains the serial device ~12us after
                                # layer end, so its expand goes end-of-layer
                                # (the deferred region-1 sweeps give its
                                # consumers +33us of slack); an in-layer
                                # emission would park the SP SEQ and
                                # head-block this layer's final g16 stores.
                                if L < 2 and not SAFE_BARRIERS:
                                    for q in range(2):
                                        if (b + 1 == 78 + 15 * q
                                                and ccs[q] is not None
                                                and expands[NREG * (L + 1) + q]
                                                is None):
                                            exp = nc.sync.dma_start(
                                                out=tpad[q * RSPAN:
                                                         (q + 1) * RSPAN, 0:D],
                                                in_=tabs[L + 1][
                                                    q * RSPAN:
                                                    (q + 1) * RSPAN, :])
                                            _add_dep_helper(
                                                exp.ins, ccs[q].ins, sync=True,
                                                reason="expand after collective")
                                            _add_dep_helper(
                                                exp.ins, g16_dmas[b].ins,
                                                sync=True,
                                                reason="pace expand dispatch")
                                            expands[NREG * (L + 1) + q] = exp

                if L < 2 and SAFE_BARRIERS:
                    tc.strict_bb_all_engine_barrier()
                    for q in range(NREG):
                        nc.gpsimd.collective_compute(
                            "AllGather", mybir.AluOpType.bypass,
                            replica_groups=rg,
                            ins=[gouts[L][q * QS:(q + 1) * QS, :]],
                            outs=[tabs[L + 1][q * RSPAN:(q + 1) * RSPAN, :]])
                    tc.strict_bb_all_engine_barrier()
                    for q in range(NREG):
                        nc.sync.dma_start(
                            out=tpad[q * RSPAN:(q + 1) * RSPAN, 0:D],
                            in_=tabs[L + 1][q * RSPAN:(q + 1) * RSPAN, :])
                    tc.strict_bb_all_engine_barrier()
                elif L < 2:
                    # Next layer's table expands. DMA waits execute while
                    # holding the SP SEQ, and the tile scheduler orders by
                    # deps alone - so chain each expand behind a g16 store
                    # ~30 block-epilogues after its quarter's collective was
                    # triggered: by then the collective has completed and
                    # the expand never parks the SEQ (a park head-blocks all
                    # later SP DMA dispatches for up to 55us).
                    for q in range(NREG):
                        if expands[NREG * (L + 1) + q] is not None:
                            continue
                        exp = nc.sync.dma_start(
                            out=tpad[q * RSPAN:(q + 1) * RSPAN, 0:D],
                            in_=tabs[L + 1][q * RSPAN:(q + 1) * RSPAN, :])
                        _add_dep_helper(exp.ins, ccs[q].ins, sync=True,
                                        reason="expand after quarter collective")
                        db = min(NB - 1, nb_q[q] + 30)
                        _add_dep_helper(exp.ins, g16_dmas[db].ins, sync=True,
                                        reason="delay expand past collective")
                        expands[NREG * (L + 1) + q] = exp
    nc.finalize()
    return nc


# ------------------------------------------------------------------- kernel

_CACHE = {}


def _edge_key(edge_index):
    e = np.asarray(edge_index)
    import hashlib
    h = hashlib.md5()
    h.update(str(e.shape).encode())
    h.update(np.ascontiguousarray(e[:, ::997]).tobytes())
    h.update(np.ascontiguousarray(e[:, -7:]).tobytes())
    return h.hexdigest()


def _get_plan(N, edge_index, fc_b_val):
    key = (_edge_key(edge_index), N, round(float(fc_b_val), 9))
    if key not in _CACHE:
        struct, pcd, dinv, perms = _preprocess(N, edge_index)
        nc = _build_program(struct, fc_b_val)
        _CACHE.clear()
        _CACHE[key] = (struct, pcd, dinv, perms, nc)
    return _CACHE[key]


def kernel(x1, edge_index1, W11, b11, W12, b12, W13, b13, fc_w, fc_b):
    from concourse.bass_utils import run_bass_kernel_spmd

    x1 = np.asarray(x1, np.float32)
    edge_index = np.asarray(edge_index1)
    fc_b_val = float(np.asarray(fc_b).reshape(-1)[0])
    struct, pcd, dinv, perms, nc = _get_plan(x1.shape[0], edge_index, fc_b_val)
    t1c = _host_tables(x1, W11, dinv, perms, struct)
    t1pad = np.zeros((struct["TBL"], ELEM), BF16)
    t1pad[:, :D] = t1c

    NB, NSH, NLOC, QS, RSPAN = (struct["NB"], struct["NSH"], struct["NLOC"],
                                struct["QS"], struct["RSPAN"])

    iota = np.tile(np.repeat(np.arange(P, dtype=np.float32), KSEL)[None, :],
                   (P, 1)).astype(BF16)
    bcol = np.stack([np.asarray(b11, np.float32),
                     np.asarray(b12, np.float32),
                     np.asarray(b13, np.float32)], axis=1)   # [D, 3]

    in_maps = []
    for c in range(NCORES):
        dinv_loc = np.zeros(NLOC, np.float32)
        dinv_loc[:NSH] = dinv[c * NSH:(c + 1) * NSH][perms[c]]
        # own-shard layer-1 gown rows = dinv * t1, block-major [P, NB*D]
        own = np.ascontiguousarray(
            t1c.reshape(NREG, NCORES, QS, D)[:, c].astype(np.float32)
        ).reshape(NLOC, D) * dinv_loc[:, None]
        own = own.reshape(NB, P, D).transpose(1, 0, 2).reshape(P, NB * D)
        in_maps.append({
            "t1pad": t1pad,
            "idx": np.tile(pcd["idx"][c], (8, 1)),
            "dstloc": pcd["dstloc"][c].astype(BF16),
            "dinvb": dinv_loc.reshape(NB, P).T.copy(),
            "g1own": np.ascontiguousarray(own).astype(BF16),
            "bcol": bcol,
            "w2": np.asarray(W12, np.float32).astype(BF16),
            "w3": np.asarray(W13, np.float32).astype(BF16),
            "fcw": np.asarray(fc_w, np.float32).astype(BF16),
            "iota": iota,
        })

    res = run_bass_kernel_spmd(nc, in_maps, core_ids=list(range(NCORES)))

    out = np.zeros((struct["N"], 1), np.float32)
    for c in range(NCORES):
        yc = res.results[c]["y"][:NSH, 0]
        out[c * NSH + perms[c], 0] = yc
    return out



# revision 111
# speedup vs baseline: 1.0442x; 1.0442x over previous
"""3-layer GCN (PyG GCNConv semantics) on 8 Trainium2 NeuronCores.

Design (v5, 0.898 ms TimelineSim vs 1.62 ms v2):
  - The first 36 blocks' region-1 sweeps are deferred to the end of phase
    A (region-0 partial stashed alone, deferred sweep accumulates into the
    stash): region 1 of the table is first gathered ~33us into the layer,
    absorbing the previous layer's quarter-1 AllGather latency (it drains
    the serial collective device ~12us after that layer ends). Cuts the
    two ~42us layer-boundary stalls for one extra DVE op per deferred
    block. Expand q1 is emitted in-layer at block 93 (q0 at 78): late
    enough that its SP SEQ park is short, early enough to beat
    end-of-layer dispatch (both alternatives measured slower).

From v4 (0.935 ms):
  - Sel one-hot masks built KSEL=32 tiles per DVE instruction: d-major
    interleaved layout sel[p, d*K+j] compared against a 0-stride broadcast
    of the dstloc columns. The broadcast sits on a middle AP dim so the
    2-byte packed DVE fast mode still applies: ~69ns/tile vs 93, and 32x
    fewer DVE instructions (fewer sem hops, less PE Ldweights wait time).
    The per-tile matmul lhsT is a stride-K column view of the group mask
    (verified bit-exact through the device path).

Inherited from v3 (1.21 ms):
  - Table rows quarter-mapped: region r of the gather table = quarter r of
    every core's shard. The inter-layer AllGather is split into 4 per-region
    collectives; each next-layer expand DMA is chained behind BOTH its
    quarter's collective AND a g16 store ~30 block-epilogues later, so it
    reaches the SP sequencer with its wait already satisfied (a parked DMA
    wait head-blocks all later SP DMA dispatches for up to 55us). The
    layer-1 table arrives pre-padded from the host: no startup expands.
  - EGATHER=32: 64B gather descriptors at the 7ns DMA floor (raw
    InstDMAGatherAnt; 256B-multiple restriction is transpose-only;
    verified bit-exact on hardware).
  - Two-phase layers over groups of GBLK=6 dst blocks: phase A sweeps src
    regions {0,1} region-minor, each block's partial accumulating in one
    PSUM bank, then one DVE op stashes dinv*gpA+gown to SBUF (bf16);
    phase B sweeps regions {2,3} the same way and finishes with the
    epilogue. Regions 2,3 of the table are first needed ~50% into the
    layer, giving the previous layer's late collectives half a layer of
    slack; PSUM residency removes all per-region acc copy/add DVE ops.
  - Epilogue: xb = dinv*gpB + stash on DVE, PE transpose, relu(x+b) and
    the g16/gown rescales on the idle Activation engine; g16 lives in a
    persistent buffer that doubles as next layer's gown source.
  - Deep pools (sel=4x32 tiles, msg=10 calls) so the sel stream rides ahead
    of the PE.
  - Remaining bottlenecks: DVE busy ~860us (88%), PE SEQ issue rate
    (~80ns/Ldweights, ~100% held), and 2x ~42us layer-boundary stalls from
    the quarter-0/1 collectives serializing on COLLECTIVE_CORES (55us
    each) past the layer end. Going lower needs half-quarter collectives
    (restructured expand APs) or fewer tiles; int16 gather indices cap
    regions at 32K rows (interp asserts idx >= -1, so no signed trick).
"""

import numpy as np
import ml_dtypes

P = 128
D = 32             # feature width
ELEM = 128         # table row stride: 128 bf16 = 256B
EGATHER = 32       # gathered elems per row (32 -> 64B descriptors @ 7ns floor)
SAFE_BARRIERS = False  # True: barrier-fenced boundaries (debug), no sem pipeline
MAX_CALL = 8192    # max indices per dma_gather call
NCORES = 8
NREG = 4           # src index regions (int16 reach); also collective quarters
GBLK = 6           # dst blocks per PSUM-resident accumulation group
KSEL = 32          # sel masks built per DVE instruction (batched is_equal)

BF16 = ml_dtypes.bfloat16


# ----------------------------------------------------------------- host side

def _preprocess(N, edge_index):
    """Edge structure only (no x-dependent data): cacheable."""
    assert N % NCORES == 0
    NSH = N // NCORES                       # dst nodes per core
    NLOC = ((NSH + P - 1) // P) * P         # padded to blocks of 128
    NB = NLOC // P
    assert NLOC % NREG == 0
    QS = NLOC // NREG                       # quarter size (shard rows)
    TBL = NLOC * NCORES                     # total table rows
    RSPAN = TBL // NREG                     # table rows per region
    assert RSPAN <= 32767

    src = np.asarray(edge_index[0], dtype=np.int64)
    dst = np.asarray(edge_index[1], dtype=np.int64)
    deg = np.bincount(dst, minlength=N).astype(np.float64) + 1.0
    dinv = (1.0 / np.sqrt(deg)).astype(np.float32)

    core_of = dst // NSH

    # per-core permutation (in-degree desc within shard); table row of node
    # (core c, slot s): quarter q = s // QS -> q*RSPAN + c*QS + s%QS
    perms, invperms = [], []
    g2t = np.empty(N, np.int64)
    dcnt_all = np.bincount(dst, minlength=N) + 1   # incl self-loop
    for c in range(NCORES):
        cnt = dcnt_all[c * NSH:(c + 1) * NSH]
        perm = np.argsort(-cnt, kind="stable")     # slot -> local node
        inv = np.empty(NSH, np.int64)
        inv[perm] = np.arange(NSH)
        perms.append(perm)
        invperms.append(inv)
        q = inv // QS
        g2t[c * NSH:(c + 1) * NSH] = q * RSPAN + c * QS + inv % QS

    # per-core edge lists (NO self-loops); keyed by (region, dst-block)
    per_core = []
    for c in range(NCORES):
        m = core_of == c
        s_c = src[m]
        dslot = invperms[c][dst[m] - c * NSH]
        stid = g2t[s_c]
        reg = stid // RSPAN
        blk = dslot // P
        per_core.append((stid, dslot, reg, blk))

    # common tile structure: tiles_rb[r, b] = max over cores
    counts = np.zeros((NCORES, NREG, NB), np.int64)
    for c in range(NCORES):
        _, _, reg, blk = per_core[c]
        np.add.at(counts[c], (reg, blk), 1)
    tiles_rb = np.maximum((counts.max(axis=0) + P - 1) // P, 1)  # [NREG, NB]

    # Two-phase, group-of-G ordering. Phase A sweeps regions {0,1} over
    # groups of G blocks (region-minor); each block's partial lives in one
    # PSUM bank for its group sweep, then one DVE op stashes
    # dinv*partial+gown to SBUF. Phase B sweeps regions {2,3} the same way
    # and finishes with the epilogue.
    #
    # The first DEFG blocks' region-1 sweeps are DEFERRED to the end of
    # phase A (their region-0 partial is stashed on its own, and the
    # deferred region-1 sweep accumulates into the stash with one extra
    # DVE op per block). Region 1 of the table is then first gathered
    # ~DEFG*region-tiles into the layer instead of ~one group in - enough
    # slack for the previous layer's quarter-1 AllGather (which drains the
    # serial collective device only ~20us after that layer ends) to land
    # without stalling the pipe. Regions 2,3 are first needed at ~50%.
    #
    # sess[(r, b)] = (session_start, session_stop, stop_kind) where
    # stop_kind 0 = stash-write, 1 = stash-accumulate, 2 = epilogue.
    DEFG = 7 * GBLK
    tile_meta = []        # (region, block, bucket_first, bucket_last)
    sess = {}

    def emit_bucket(r, b):
        nt = int(tiles_rb[r, b])
        for i in range(nt):
            tile_meta.append((r, b, i == 0, i == nt - 1))

    for g0 in range(0, DEFG, GBLK):          # part 1: r0 only, stash-write
        for b in range(g0, g0 + GBLK):
            emit_bucket(0, b)
            sess[(0, b)] = (True, True, 0)
    for g0 in range(DEFG, NB, GBLK):         # part 2: r0+r1, stash-write
        for r in (0, 1):
            for b in range(g0, min(g0 + GBLK, NB)):
                emit_bucket(r, b)
                sess[(r, b)] = (r == 0, r == 1, 0)
    for g0 in range(0, DEFG, GBLK):          # part 3: deferred r1, stash-acc
        for b in range(g0, g0 + GBLK):
            emit_bucket(1, b)
            sess[(1, b)] = (True, True, 1)
    for g0 in range(0, NB, GBLK):            # phase B: r2+r3, epilogue
        for r in (2, 3):
            for b in range(g0, min(g0 + GBLK, NB)):
                emit_bucket(r, b)
                sess[(r, b)] = (r == 2, r == 3, 2)
    T = len(tile_meta)

    # gather calls: chunk tile stream, never crossing region boundaries
    calls = []            # (region, tile_start, ntiles)
    t0 = 0
    while t0 < T:
        r = tile_meta[t0][0]
        nt = 1
        while (t0 + nt < T and tile_meta[t0 + nt][0] == r
               and nt < MAX_CALL // P):
            nt += 1
        calls.append((r, t0, nt))
        t0 += nt

    idx_cols = sum(cl[2] * P // 16 for cl in calls)
    # per-call idx-buffer assignment: the idx table is split into NREG
    # SBUF tiles by stream position (preload pipelining); assignment is
    # monotone and calls never straddle a split.
    call_sb = []
    for (r, t0c, nt) in calls:
        sbi = min(NREG - 1, NREG * t0c // T)
        if call_sb and sbi < call_sb[-1]:
            sbi = call_sb[-1]
        call_sb.append(sbi)
    sb_col_base = [0] * (NREG + 1)
    col = 0
    for ci, (r, t0c, nt) in enumerate(calls):
        col += nt * P // 16
        sb_col_base[call_sb[ci] + 1] = col
    idx_all = np.zeros((NCORES, 16, idx_cols), np.int16)
    dl_all = np.full((NCORES, P, T), 255.0, np.float32)   # 255 = dead slot

    pos = {}
    for ti, (r, b, gf, _gl) in enumerate(tile_meta):
        if gf:
            pos[(r, b)] = ti

    for c in range(NCORES):
        stid, dsl, reg, blk = per_core[c]
        eidx = np.zeros((T, P), np.int64)
        dloc = np.full((T, P), 255, np.int64)
        for ti, (r, _b, _f, _l) in enumerate(tile_meta):
            eidx[ti, :] = r * RSPAN                # any finite row in region
        key = reg * NB + blk
        order = np.argsort(key, kind="stable")
        ks = key[order]
        st_ids = stid[order]
        dls = dsl[order]
        uq, starts = np.unique(ks, return_index=True)
        starts = list(starts) + [len(ks)]
        for u_i, k in enumerate(uq):
            r, b = int(k) // NB, int(k) % NB
            lo, hi = starts[u_i], starts[u_i + 1]
            n = hi - lo
            ti = pos[(r, b)] + np.arange(n) // P
            lane = np.arange(n) % P
            eidx[ti, lane] = st_ids[lo:hi]
            dloc[ti, lane] = dls[lo:hi] % P
        dl_all[c] = dloc.T.astype(np.float32)
        col0 = 0
        for (r, t0c, nt) in calls:
            flat = (eidx[t0c:t0c + nt].reshape(-1) - r * RSPAN).astype(np.int16)
            ncol = nt * P // 16
            idx_all[c, :, col0:col0 + ncol] = flat.reshape(ncol, 16).T
            col0 += ncol

    struct = {
        "N": N, "NSH": NSH, "NLOC": NLOC, "NB": NB, "QS": QS, "TBL": TBL,
        "RSPAN": RSPAN, "tile_meta": tile_meta, "calls": calls, "T": T,
        "idx_cols": idx_cols,
        "call_sb": call_sb, "sb_col_base": sb_col_base, "sess": sess,
    }
    per_core_data = {"idx": idx_all, "dstloc": dl_all}
    return struct, per_core_data, dinv, perms


def _host_tables(x1, W11, dinv, perms, struct):
    """layer-1 compact table t1c = (dinv*x1) @ W11, quarter-mapped, bf16."""
    NSH, NLOC, QS, RSPAN = (struct["NSH"], struct["NLOC"], struct["QS"],
                            struct["RSPAN"])
    g1 = (dinv[:, None] * np.asarray(x1, np.float32)) @ np.asarray(W11, np.float32)
    t1 = np.zeros((struct["TBL"], D), np.float32)
    for c in range(NCORES):
        gperm = np.zeros((NLOC, D), np.float32)
        gperm[:NSH] = g1[c * NSH + perms[c]]
        # shard slot s -> row (s//QS)*RSPAN + c*QS + s%QS
        t1.reshape(NREG, NCORES, QS, D)[:, c] = gperm.reshape(NREG, QS, D)
    return t1.astype(BF16)


# --------------------------------------------------------------- device side

def _raw_dma_gather(g, out_ap, in_ap, idxs_ap, num_idxs, elem_size, elem_step,
                    queue_num=0):
    """BassGpSimd.dma_gather minus the %256 elem_size restriction."""
    import concourse.bass as bass
    import concourse.mybir as mybir
    from concourse import ap_utils
    assert idxs_ap.dtype == mybir.dt.int16
    assert in_ap.dtype == out_ap.dtype
    assert ap_utils.ap_is_contiguous(in_ap.ap[1:])
    assert ap_utils.ap_is_contiguous(out_ap.ap[1:])
    assert ap_utils.ap_is_contiguous(idxs_ap.ap[1:])
    assert in_ap.ap[-1][1] == out_ap.ap[-1][1] == elem_size
    assert in_ap.ap[0][0] == elem_step
    stride_bytes = elem_step * mybir.dt.size(in_ap.dtype)
    stride_bytes_256 = stride_bytes // 256
    assert stride_bytes % 256 == 0 and stride_bytes_256 < 256
    _in_ap = g.lower_ap_dma(in_ap, for_custom_bir_dma=True)
    _idxs_ap = g.lower_ap(idxs_ap)
    _out_ap = g.lower_ap(out_ap)
    return g.add_instruction(
        mybir.InstDMAGatherAnt(
            name=g.bass.get_next_instruction_name(),
            ins=[*_in_ap, _idxs_ap, g.lower_val_access(g.to_reg(num_idxs))],
            outs=[_out_ap],
            transpose=False,
            num_idxs=num_idxs,
            elem_size=elem_size,
            stride_bytes_256=stride_bytes_256,
            gen_mode=0,
            single_packet=False,
            queue_num=queue_num,
        )
    )


def _build_program(struct, fc_b_val):
    import concourse.bacc as bacc
    import concourse.mybir as mybir
    import concourse.tile as tile
    from concourse.library_config import mlp
    from concourse.masks import make_identity

    NB, TBL, RSPAN, QS = (struct["NB"], struct["TBL"], struct["RSPAN"],
                          struct["QS"])
    NLOC = struct["NLOC"]
    T = struct["T"]
    tile_meta = struct["tile_meta"]
    calls = struct["calls"]
    idx_cols = struct["idx_cols"]
    sess = struct["sess"]

    nc = bacc.Bacc(None, target_bir_lowering=False, num_swdge_queues=4)
    dt = mybir.dt

    # layer-1 table arrives pre-padded to the 256B-row gather layout (host
    # pads for free): no startup expand DMAs at all.
    tpad = nc.declare_dram_parameter("t1pad", [TBL, ELEM], dt.bfloat16,
                                     isOutput=False)
    idx = nc.declare_dram_parameter("idx", [P, idx_cols], dt.int16, isOutput=False)
    dstloc = nc.declare_dram_parameter("dstloc", [P, T], dt.bfloat16, isOutput=False)
    dinvb = nc.declare_dram_parameter("dinvb", [P, NB], dt.float32, isOutput=False)
    g1own = nc.declare_dram_parameter("g1own", [P, NB * D], dt.bfloat16, isOutput=False)
    bcol = nc.declare_dram_parameter("bcol", [D, 3], dt.float32, isOutput=False)
    w2 = nc.declare_dram_parameter("w2", [D, D], dt.bfloat16, isOutput=False)
    w3 = nc.declare_dram_parameter("w3", [D, D], dt.bfloat16, isOutput=False)
    fcw = nc.declare_dram_parameter("fcw", [D, 1], dt.bfloat16, isOutput=False)
    iota = nc.declare_dram_parameter("iota", [P, P * KSEL], dt.bfloat16,
                                     isOutput=False)
    y = nc.declare_dram_parameter("y", [NLOC, 1], dt.float32, isOutput=True)

    g2c = nc.dram_tensor("g2c", [NLOC, D], dt.bfloat16)
    g3c = nc.dram_tensor("g3c", [NLOC, D], dt.bfloat16)
    t2c = nc.dram_tensor("t2c", [TBL, D], dt.bfloat16, addr_space="Shared")
    t3c = nc.dram_tensor("t3c", [TBL, D], dt.bfloat16, addr_space="Shared")

    rg = [list(range(NCORES))]
    CHUNKS = MAX_CALL // P

    # blocks that must be written before quarter q's collective fires
    nb_q = [-(-((q + 1) * QS) // P) for q in range(NREG)]     # cumulative

    from concourse.bass import _add_dep_helper

    with tile.TileContext(nc) as tc:
        with (
            tc.tile_pool(name="const", bufs=1) as cpool,
            tc.tile_pool(name="msg", bufs=(10 if EGATHER <= 32 else 5)) as mpool,
            tc.tile_pool(name="sel", bufs=4) as spool,
            tc.tile_pool(name="ep", bufs=16) as epool,
            tc.tile_pool(name="stash", bufs=2) as stashpool,
            tc.tile_pool(name="gsb", bufs=2) as gsbpool,
            tc.tile_pool(name="gp", bufs=GBLK, space="PSUM") as gpool,
            tc.tile_pool(name="eppsum", bufs=1, space="PSUM") as eppool,
        ):
            nc.gpsimd.load_library(mlp)
            call_sb = struct["call_sb"]
            sb_col_base = struct["sb_col_base"]
            idx_sbs = [cpool.tile([P, sb_col_base[i + 1] - sb_col_base[i]],
                                  dt.int16, name=f"idxsb{i}")
                       for i in range(NREG)]
            dl_sb = cpool.tile([P, T], dt.bfloat16)
            dinv_sb = cpool.tile([P, NB], dt.float32)
            g1own_sb = cpool.tile([P, NB * D], dt.bfloat16)
            bcol_sb = cpool.tile([D, 3], dt.float32)
            w2_sb = cpool.tile([D, D], dt.bfloat16)
            w3_sb = cpool.tile([D, D], dt.bfloat16)
            fcw_sb = cpool.tile([D, 1], dt.bfloat16)
            iota_sb = cpool.tile([P, P * KSEL], dt.bfloat16)
            ident = cpool.tile([P, P], dt.bfloat16)

            # startup order: what the first gathers need comes first. The
            # first gather call and sel batches only touch the leading
            # idx/dstloc columns, so those land as small chunks before the
            # bulk loads (slice-level deps let consumers start early).
            nc.sync.dma_start(out=iota_sb[:], in_=iota[:])
            SP0 = min(512, sb_col_base[1])
            nc.sync.dma_start(out=idx_sbs[0][:, :SP0], in_=idx[:, :SP0])
            nc.sync.dma_start(out=dl_sb[:, :256], in_=dstloc[:, :256])
            nc.sync.dma_start(out=idx_sbs[0][:, SP0:],
                              in_=idx[:, SP0:sb_col_base[1]])
            nc.sync.dma_start(out=dl_sb[:, 256:], in_=dstloc[:, 256:])
            make_identity(nc, ident[:])
            # layer-1 table is pre-padded on host: no startup expands
            expands = [None] * (3 * NREG)
            idx_loads = [None] * NREG
            for i in range(1, NREG):
                idx_loads[i] = nc.sync.dma_start(
                    out=idx_sbs[i][:],
                    in_=idx[:, sb_col_base[i]:sb_col_base[i + 1]])
            nc.sync.dma_start(out=dinv_sb[:], in_=dinvb[:])
            nc.sync.dma_start(out=g1own_sb[:], in_=g1own[:])
            nc.sync.dma_start(out=bcol_sb[:], in_=bcol[:])
            nc.sync.dma_start(out=w2_sb[:], in_=w2[:])
            nc.sync.dma_start(out=w3_sb[:], in_=w3[:])
            nc.sync.dma_start(out=fcw_sb[:], in_=fcw[:])
            if SAFE_BARRIERS:
                tc.strict_bb_all_engine_barrier()

            tabs = [None, t2c, t3c]
            gouts = [g2c, g3c, None]
            wnext = [w2_sb, w3_sb, None]

            def emit_epilogue(L, b, gp, gcur, stash):
                # conv_out = dinv*gpB + (dinv*gpA + gown) [phase-A stash]
                xb = epool.tile([P, D], dt.bfloat16, name=f"x{L}_{b}", tag="xb")
                nc.vector.scalar_tensor_tensor(
                    out=xb[:], in0=gp[:],
                    scalar=dinv_sb[:, b:b + 1],
                    in1=stash[:, b * D:(b + 1) * D],
                    op0=mybir.AluOpType.mult, op1=mybir.AluOpType.add)
                xT = eppool.tile([D, P], dt.bfloat16, name=f"xT{L}_{b}", tag="xT")
                nc.tensor.transpose(out=xT[:], in_=xb[:], identity=ident[:])
                xT_sb = epool.tile([D, P], dt.bfloat16,
                                   name=f"xTs{L}_{b}", tag="xTs")
                nc.scalar.activation(
                    out=xT_sb[:], in_=xT[:],
                    func=mybir.ActivationFunctionType.Relu,
                    bias=bcol_sb[:, L:L + 1], scale=1.0)
                if L < 2:
                    h = eppool.tile([P, D], dt.float32,
                                    name=f"h{L}_{b}", tag="h")
                    nc.tensor.matmul(out=h[:], lhsT=xT_sb[:],
                                     rhs=wnext[L][:], start=True, stop=True)
                    # g16 = dinv*h: collective input. The next layer's
                    # self-loop term gown = dinv*g16 goes to the persistent
                    # gown buffer via one more cheap Act rescale.
                    nc.scalar.activation(
                        out=gcur[:, b * D:(b + 1) * D], in_=h[:],
                        func=mybir.ActivationFunctionType.Copy,
                        scale=dinv_sb[:, b:b + 1])
                    gdma = nc.sync.dma_start(
                        out=gouts[L][b * P:(b + 1) * P, :],
                        in_=gcur[:, b * D:(b + 1) * D])
                    g16_dmas[b] = gdma
                    nc.scalar.activation(
                        out=gownbuf[:, b * D:(b + 1) * D],
                        in_=gcur[:, b * D:(b + 1) * D],
                        func=mybir.ActivationFunctionType.Copy,
                        scale=dinv_sb[:, b:b + 1])
                else:
                    yp = eppool.tile([P, 1], dt.float32,
                                     name=f"yp{b}", tag="h")
                    nc.tensor.matmul(out=yp[:], lhsT=xT_sb[:],
                                     rhs=fcw_sb[:], start=True, stop=True)
                    y_sb = epool.tile([P, 1], dt.float32,
                                      name=f"ys{b}", tag="g")
                    nc.vector.tensor_scalar(
                        out=y_sb[:], in0=yp[:],
                        scalar1=float(fc_b_val), scalar2=None,
                        op0=mybir.AluOpType.add)
                    nc.sync.dma_start(out=y[b * P:(b + 1) * P, :], in_=y_sb[:])

            gownbuf = gsbpool.tile([P, NB * D], dt.bfloat16,
                                   name="gownbuf", tag="gown")
            for L in range(3):
                next_q = 0            # next quarter collective to emit
                ccs = [None] * NREG
                g16_dmas = [None] * NB
                gp_of = {}            # block -> live PSUM accumulator tile
                gown = g1own_sb if L == 0 else gownbuf
                stash = stashpool.tile([P, NB * D], dt.bfloat16,
                                       name=f"stash{L}", tag="stash")
                gcur = (gsbpool.tile([P, NB * D], dt.bfloat16,
                                     name=f"gcur{L}", tag="gsb")
                        if L < 2 else None)
                icol = 0
                for ci, (r, t0c, nt) in enumerate(calls):
                    dep_exp = None if SAFE_BARRIERS else expands[NREG * L + r]
                    nidx = nt * P
                    ncol = nidx // 16
                    sbi = call_sb[ci]
                    lcol = icol - sb_col_base[sbi]
                    msg = mpool.tile([P, CHUNKS * EGATHER], dt.bfloat16,
                                     name=f"msg{L}_{ci}", tag="msg")
                    if EGATHER == ELEM:
                        gi = nc.gpsimd.dma_gather(
                            msg[:, : nt * EGATHER].rearrange(
                                "p (c e) -> p c e", e=EGATHER),
                            tpad[r * RSPAN:(r + 1) * RSPAN, :],
                            idx_sbs[sbi][:, lcol:lcol + ncol],
                            nidx, nidx, EGATHER,
                            single_packet=False, queue_num=ci % 4)
                    else:
                        gi = _raw_dma_gather(
                            nc.gpsimd,
                            msg[:, : nt * EGATHER].rearrange(
                                "p (c e) -> p c e", e=EGATHER),
                            tpad[r * RSPAN:(r + 1) * RSPAN, 0:EGATHER],
                            idx_sbs[sbi][:, lcol:lcol + ncol],
                            nidx, EGATHER, ELEM, queue_num=ci % 4)
                    if dep_exp is not None:
                        _add_dep_helper(gi.ins, dep_exp.ins, sync=True,
                                        reason="gather after region expand")
                    if L == 0 and ci in (2, 4, 6):
                        # startup: later idx loads are not needed for a
                        # while; chain them behind early gathers so they
                        # don't hog the DMA engines before the pipeline is
                        # primed.
                        _add_dep_helper(idx_loads[ci // 2].ins, gi.ins,
                                        sync=True, reason="delay idx load")
                    icol += ncol
                    msg3 = msg[:].rearrange("p (c e) -> p c e", e=EGATHER)
                    for c in range(nt):
                        t_glob = t0c + c
                        _r, b, gfst, glst = tile_meta[t_glob]
                        # sel masks built KSEL tiles per DVE instruction in
                        # d-major interleave: sel_g[p, d*kb + j] is mask
                        # element (edge p, dst d) of tile t0+j. The 0-stride
                        # broadcast of dl sits on a middle AP dim, keeping
                        # the 2-byte packed fast mode: ~74ns/tile vs 93,
                        # and 8x fewer DVE instructions.
                        if t_glob % KSEL == 0:
                            kb = min(KSEL, T - t_glob)
                            sel_g = spool.tile([P, P * kb], dt.bfloat16,
                                               name=f"sel{L}_{t_glob}",
                                               tag="sel")
                            i3 = iota_sb[:].rearrange(
                                "p (d k) -> p d k", k=KSEL)[:, :, :kb]
                            d3 = dl_sb[:, t_glob:t_glob + kb].rearrange(
                                "p (o k) -> p o k", o=1).broadcast_to(
                                (P, P, kb))
                            nc.vector.tensor_tensor(
                                out=sel_g[:].rearrange(
                                    "p (d k) -> p d k", k=kb),
                                in0=i3, in1=d3,
                                op=mybir.AluOpType.is_equal)
                            sel_view = sel_g[:].rearrange(
                                "p (d k) -> p d k", k=kb)
                        sstart, sstop, skind = sess[(_r, b)]
                        if sstart and gfst:
                            gp_of[b] = gpool.tile([P, D], dt.float32,
                                                  name=f"gp{L}_{t_glob}",
                                                  tag="gp")
                        gp = gp_of[b]
                        nc.tensor.matmul(
                            out=gp[:],
                            lhsT=sel_view[:, :, t_glob % KSEL],
                            rhs=msg3[:, c, 0:D],
                            start=bool(sstart and gfst),
                            stop=bool(sstop and glst),
                            skip_group_check=True)
                        if glst and sstop:
                            del gp_of[b]
                            if skind == 0:
                                # stash dinv*gp + gown to SBUF
                                nc.vector.scalar_tensor_tensor(
                                    out=stash[:, b * D:(b + 1) * D], in0=gp[:],
                                    scalar=dinv_sb[:, b:b + 1],
                                    in1=gown[:, b * D:(b + 1) * D],
                                    op0=mybir.AluOpType.mult,
                                    op1=mybir.AluOpType.add)
                            elif skind == 1:
                                # deferred region-1 partial: accumulate
                                # into the existing stash
                                nc.vector.scalar_tensor_tensor(
                                    out=stash[:, b * D:(b + 1) * D], in0=gp[:],
                                    scalar=dinv_sb[:, b:b + 1],
                                    in1=stash[:, b * D:(b + 1) * D],
                                    op0=mybir.AluOpType.mult,
                                    op1=mybir.AluOpType.add)
                            else:
                                # block b fully aggregated: epilogue inline so
                                # quarter collectives fire during phase B
                                emit_epilogue(L, b, gp, gcur, stash)
                                if (L < 2 and next_q < NREG
                                        and b + 1 == nb_q[next_q]
                                        and not SAFE_BARRIERS):
                                    cc = nc.gpsimd.collective_compute(
                                        "AllGather", mybir.AluOpType.bypass,
                                        replica_groups=rg,
                                        ins=[gouts[L][next_q * QS:
                                                      (next_q + 1) * QS, :]],
                                        outs=[tabs[L + 1][next_q * RSPAN:
                                                          (next_q + 1) * RSPAN, :]])
                                    lo = 0 if next_q == 0 else nb_q[next_q - 1]
                                    for bb in range(lo, nb_q[next_q]):
                                        _add_dep_helper(
                                            cc.ins, g16_dmas[bb].ins, sync=True,
                                            reason="collective after quarter g16")
                                    ccs[next_q] = cc
                                    next_q += 1
                                # quarter 0 of the next layer's table is
                                # needed right at the next layer's start:
                                # emit its expand in-layer at ~80% of the
                                # block stream (cc0 has just completed),
                                # paced by this block's g16 so the SP SEQ
                                # never parks on it. Quarter 1's collective
                                # only drains the serial device ~12us after
                                # layer end, so its expand goes end-of-layer
                                # (the deferred region-1 sweeps give its
                                # consumers +33us of slack); an in-layer
                                # emission would park the SP SEQ and
                                # head-block this layer's final g16 stores.
                                if L < 2 and not SAFE_BARRIERS:
                                    for q in range(2):
                                        if (b + 1 == 78 + 15 * q
                                                and ccs[q] is not None
                                                and expands[NREG * (L + 1) + q]
                                                is None):
                                            exp = nc.sync.dma_start(
                                                out=tpad[q * RSPAN:
                                                         (q + 1) * RSPAN, 0:D],
                                                in_=tabs[L + 1][
                                                    q * RSPAN:
                                                    (q + 1) * RSPAN, :])
                                            _add_dep_helper(
                                                exp.ins, ccs[q].ins, sync=True,
                                                reason="expand after collective")
                                            _add_dep_helper(
                                                exp.ins, g16_dmas[b].ins,
                                                sync=True,
                                                reason="pace expand dispatch")
                                            expands[NREG * (L + 1) + q] = exp

                if L < 2 and SAFE_BARRIERS:
                    tc.strict_bb_all_engine_barrier()
                    for q in range(NREG):
                        nc.gpsimd.collective_compute(
                            "AllGather", mybir.AluOpType.bypass,
                            replica_groups=rg,
                            ins=[gouts[L][q * QS:(q + 1) * QS, :]],
                            outs=[tabs[L + 1][q * RSPAN:(q + 1) * RSPAN, :]])
                    tc.strict_bb_all_engine_barrier()
                    for q in range(NREG):
                        nc.sync.dma_start(
                            out=tpad[q * RSPAN:(q + 1) * RSPAN, 0:D],
                            in_=tabs[L + 1][q * RSPAN:(q + 1) * RSPAN, :])
                    tc.strict_bb_all_engine_barrier()
                elif L < 2:
                    # Next layer's table expands. DMA waits execute while
                    # holding the SP SEQ, and the tile scheduler orders by
                    # deps alone - so chain each expand behind a g16 store
                    # ~30 block-epilogues after its quarter's collective was
                    # triggered: by then the collective has completed and
                    # the expand never parks the SEQ (a park head-blocks all
                    # later SP DMA dispatches for up to 55us).
                    for q in range(NREG):
                        if expands[NREG * (L + 1) + q] is not None:
                            continue
                        exp = nc.sync.dma_start(
                            out=tpad[q * RSPAN:(q + 1) * RSPAN, 0:D],
                            in_=tabs[L + 1][q * RSPAN:(q + 1) * RSPAN, :])
                        _add_dep_helper(exp.ins, ccs[q].ins, sync=True,
                                        reason="expand after quarter collective")
                        db = min(NB - 1, nb_q[q] + 30)
                        _add_dep_helper(exp.ins, g16_dmas[db].ins, sync=True,
                                        reason="delay expand past collective")
                        expands[NREG * (L + 1) + q] = exp
    nc.finalize()
    return nc


# ------------------------------------------------------------------- kernel

_CACHE = {}


def _edge_key(edge_index):
    e = np.asarray(edge_index)
    import hashlib
    h = hashlib.md5()
    h.update(str(e.shape).encode())
    h.update(np.ascontiguousarray(e[:, ::997]).tobytes())
    h.update(np.ascontiguousarray(e[:, -7:]).tobytes())
    return h.hexdigest()


def _get_plan(N, edge_index, fc_b_val):
    key = (_edge_key(edge_index), N, round(float(fc_b_val), 9))
    if key not in _CACHE:
        struct, pcd, dinv, perms = _preprocess(N, edge_index)
        nc = _build_program(struct, fc_b_val)
        _CACHE.clear()
        _CACHE[key] = (struct, pcd, dinv, perms, nc)
    return _CACHE[key]


def kernel(x1, edge_index1, W11, b11, W12, b12, W13, b13, fc_w, fc_b):
    from concourse.bass_utils import run_bass_kernel_spmd

    x1 = np.asarray(x1, np.float32)
    edge_index = np.asarray(edge_index1)
    fc_b_val = float(np.asarray(fc_b).reshape(-1)[0])
    struct, pcd, dinv, perms, nc = _get_plan(x1.shape[0], edge_index, fc_b_val)
    t1c = _host_tables(x1, W11, dinv, perms, struct)
    t1pad = np.zeros((struct["TBL"], ELEM), BF16)
    t1pad[:, :D] = t1c

    NB, NSH, NLOC, QS, RSPAN = (struct["NB"], struct["NSH"], struct["NLOC"],
                                struct["QS"], struct["RSPAN"])

    iota = np.tile(np.repeat(np.arange(P, dtype=np.float32), KSEL)[None, :],
                   (P, 1)).astype(BF16)
    bcol = np.stack([np.asarray(b11, np.float32),
                     np.asarray(b12, np.float32),
                     np.asarray(b13, np.float32)], axis=1)   # [D, 3]

    in_maps = []
    for c in range(NCORES):
        dinv_loc = np.zeros(NLOC, np.float32)
        dinv_loc[:NSH] = dinv[c * NSH:(c + 1) * NSH][perms[c]]
        # own-shard layer-1 gown rows = dinv * t1, block-major [P, NB*D]
        own = np.ascontiguousarray(
            t1c.reshape(NREG, NCORES, QS, D)[:, c].astype(np.float32)
        ).reshape(NLOC, D) * dinv_loc[:, None]
        own = own.reshape(NB, P, D).transpose(1, 0, 2).reshape(P, NB * D)
        in_maps.append({
            "t1pad": t1pad,
            "idx": np.tile(pcd["idx"][c], (8, 1)),
            "dstloc": pcd["dstloc"][c].astype(BF16),
            "dinvb": dinv_loc.reshape(NB, P).T.copy(),
            "g1own": np.ascontiguousarray(own).astype(BF16),
            "bcol": bcol,
            "w2": np.asarray(W12, np.float32).astype(BF16),
            "w3": np.asarray(W13, np.float32).astype(BF16),
            "fcw": np.asarray(fc_w, np.float32).astype(BF16),
            "iota": iota,
        })

    res = run_bass_kernel_spmd(nc, in_maps, core_ids=list(range(NCORES)))

    out = np.zeros((struct["N"], 1), np.float32)
    for c in range(NCORES):
        yc = res.results[c]["y"][:NSH, 0]
        out[c * NSH + perms[c], 0] = yc
    return out

